# revision 42
# baseline (speedup 1.0000x reference)
"""DeepSpeed self-attention layer on 8 Trainium2 NeuronCores.

Sharding: tensor-parallel over heads (2 heads/core), DeepSpeed-mp style.

v2 design vs the previous baseline:
- fp16 data everywhere (x, weights, q/k/v, probs, ctx), fp32 PSUM/stats.
- x^T loaded via the DMA-transpose XBAR directly from DRAM (kills all
  PE transposes of xn and the PSUM->SBUF staging copies).
- LayerNorm fully folded into the QKV gemm: gemm runs on raw x^T; a
  rank-2 correction matmul ([-mu; sqrt(var+eps)] x [wsum; bias]) rides
  the same PSUM accumulation, and the 1/sqrt(var+eps) factor is applied
  by the rotary scalar_tensor_tensor ops (q,k) and the v-copy scale.
- rotary on q and k in one fused pass (shared cos/sin tables; the
  1/sqrt(hd) score scale moved into the Exp activation's scale imm).
- softmax: Exp over kt-pairs (merged [128,2,512] activation), row-sum
  via the ones-row/ones-col trick, normalization by
  partition_broadcast(1/rowsum) + one tensor_tensor multiply.
- AllGather in fp16, output projection in fp16.
"""

import numpy as np

import concourse.bass as bass
import concourse.mybir as mybir
import concourse.tile as tile
from concourse import bacc

# Steer the act-table load-insertion pass so Ln (used for sqrt/rsqrt via
# exp(+-0.5*ln)) and the attention Exps resolve to ONE table set
# (natural_log_exp_and_others): hide exp/ln from the earlier sets the
# greedy pass would otherwise pick, which would thrash table loads.
# Set ORDER must be preserved — act_func_set_id is the index into
# act_info.json, and walrus resolves the id against the original file.
if not getattr(bacc, "_act_tables_patched", False):
    _orig_gat = bacc.get_activation_tables

    def _gat_pref(arch):
        t = {k: set(v) for k, v in _orig_gat(arch).items()}
        both = "natural_log_exp_and_others"
        if both in t:
            exp = mybir.ActivationFunctionType.Exp
            ln = mybir.ActivationFunctionType.Ln
            for name, fns in t.items():
                if name == both:
                    continue
                fns.discard(exp)
                fns.discard(ln)
        return t

    bacc.get_activation_tables = _gat_pref
    bacc._act_tables_patched = True

# Problem shape (hardcoded per contest spec)
B, S, H, NH, HD = 2, 2048, 1024, 16, 64
NCORES = 8
HPC = NH // NCORES          # heads per core = 2
T = B * S                   # 4096 flat tokens
NTILES = T // 128           # 32 token tiles
KC = H // 128               # 8 contraction chunks
TPB = S // 128              # 16 token tiles per batch
GQ = 4                      # q groups of 512 per batch
TPC = T // NCORES           # 512 tokens per core (output slice)
EPS = 1e-5
F32 = mybir.dt.float32
F32R = mybir.dt.float32r
F16 = mybir.dt.float16


def _r(ap):
    return ap.bitcast(F32R)


def _bc(ap, count, axis=1):
    """Insert a step-0 broadcast dim of size `count` at free-dim position
    `axis` (1 = right after the partition dim)."""
    new = list(ap.ap)
    new.insert(axis, [0, count])
    return bass.AP(tensor=ap.tensor, offset=ap.offset, ap=new)


def build_nc(with_cc=True):
    nc = bacc.Bacc("TRN2", num_devices=NCORES, debug=False)

    x = nc.dram_tensor("x", [T, H], F16, kind="ExternalInput")
    wqkv = nc.dram_tensor("wqkv", [H, 3 * 128], F16, kind="ExternalInput")
    corrw = nc.dram_tensor("corrw", [2, 3 * 128], F32R, kind="ExternalInput")
    cosx = nc.dram_tensor("cosx", [128, TPB, HD], F32, kind="ExternalInput")
    sinx = nc.dram_tensor("sinx", [128, TPB, HD], F32, kind="ExternalInput")
    # row 0: mask bias (pre-scaled) for kT's 65th row; row 1: ones for qT's
    kbias = nc.dram_tensor("kbias", [2, B, S], F16, kind="ExternalInput")
    ow = nc.dram_tensor("ow", [H, H], F16, kind="ExternalInput")
    identm = nc.dram_tensor("identm", [128, 128], F16, kind="ExternalInput")
    out_slice = nc.dram_tensor("out_slice", [TPC, H], F16, kind="ExternalOutput")

    with tile.TileContext(nc) as tc:
        with (
            tc.tile_pool(name="singles", bufs=1) as singles,
            tc.tile_pool(name="qkvstore", bufs=1) as qkvstore,
            tc.tile_pool(name="dram", bufs=1, space="DRAM") as dram,
        ):
            # ---- constants ----
            ident = singles.tile([128, 128], F16)
            nc.sync.dma_start(out=ident, in_=identm[:, :])
            ones1 = singles.tile([1, 128], F32)
            nc.vector.memset(ones1, 1.0)
            eps_t = singles.tile([128, 1], F32)
            nc.vector.memset(eps_t, EPS)
            wqkv_sb = singles.tile([128, KC, 384], F16)
            nc.sync.dma_start(out=wqkv_sb, in_=wqkv.rearrange("(c p) f -> p c f", p=128))
            corrw_sb = singles.tile([2, 384], F32R)
            nc.sync.dma_start(out=corrw_sb, in_=corrw[:, :])
            tabs = {}
            for name, dr in (("cos", cosx), ("sin", sinx)):
                tabs[name] = singles.tile([128, TPB, HD], F32, name=f"tab_{name}", tag=f"tab_{name}")
                nc.sync.dma_start(out=tabs[name], in_=dr[:, :, :])

            # ---- persistent per-batch q/k/v storage ----
            qT, kT, v_sb = {}, {}, {}
            for b in range(B):
                qT[b] = qkvstore.tile([65, HPC, S], F16, name=f"qT{b}")
                kT[b] = qkvstore.tile([65, HPC, S], F16, name=f"kT{b}")
                v_sb[b] = qkvstore.tile([128, TPB, HPC, 65], F16, name=f"v{b}")
                nc.gpsimd.memset(v_sb[b][:, :, :, 64:65], 1.0)
                kb_flat = bass.AP(tensor=kbias, offset=b * S, ap=[[0, 1], [0, HPC], [1, S]])
                nc.sync.dma_start(out=kT[b][64:65, :, :], in_=kb_flat)
                qo_flat = bass.AP(tensor=kbias, offset=B * S + b * S,
                                  ap=[[0, 1], [0, HPC], [1, S]])
                nc.sync.dma_start(out=qT[b][64:65, :, :], in_=qo_flat)

            ctx_local = dram.tile([HPC * HD, T], F16)
            ctx_all = dram.tile([H, T], F16)

            # persistent per-tile LN stats: corr2 rows [-mu; sqrt(var+eps)],
            # rstd = 1/sqrt(var+eps)
            corr2_all = qkvstore.tile([2, NTILES, 128], F32R, name="corr2_all")
            rstd_all = qkvstore.tile([128, NTILES], F32, name="rstd_all")

            xT_pref = {}

            def passA_tile(t, pools):
                """Stats for flat tile t: everything needing the Sqrt act
                table, done before any Exp so the table loads only twice.
                Also prefetches the tile's x^T (XBAR) two slots ahead so the
                QKV gemm never waits on the transpose DMA."""
                (xp, statp, s2_ps, xtp) = pools
                x_t = xp.tile([128, H], F16, name=f"x_{t}", tag="x")
                nc.scalar.dma_start(out=x_t, in_=x[t * 128:(t + 1) * 128, :])
                xT_t = xtp.tile([128, KC, 128], F16, name=f"xT_{t}", tag="xT")
                nc.sync.dma_start(out=xT_t, in_=x[t * 128:(t + 1) * 128, :],
                                  transpose=True)
                xT_pref[t] = xT_t
                stats = statp.tile([128, 2, 6], F16, name=f"bns_{t}", tag="bns")
                nc.vector.bn_stats(out=stats[:, 0, :], in_=x_t[:, 0:512])
                nc.vector.bn_stats(out=stats[:, 1, :], in_=x_t[:, 512:1024])
                mv = statp.tile([128, 2], F16, name=f"mv_{t}", tag="mv")
                nc.vector.bn_aggr(out=mv, in_=stats)
                # stat2 = [-mu ; sqrt(var+eps)]; sqrt/rsqrt via exp(+-0.5*ln)
                # so only the ln/exp act table is ever needed (no Sqrt-table
                # thrash against the attention Exps).
                stat2 = statp.tile([128, 2], F16, name=f"s2_{t}", tag="s2")
                nc.vector.tensor_scalar_mul(stat2[:, 0:1], mv[:, 0:1], -1.0)
                lv = statp.tile([128, 1], F32, name=f"lv_{t}", tag="lv")
                nc.scalar.activation(
                    lv, mv[:, 1:2], mybir.ActivationFunctionType.Ln,
                    bias=eps_t[:, 0:1],
                )
                nc.scalar.activation(
                    stat2[:, 1:2], lv, mybir.ActivationFunctionType.Exp, scale=0.5,
                )
                nc.scalar.activation(
                    rstd_all[:, t:t + 1], lv, mybir.ActivationFunctionType.Exp,
                    scale=-0.5,
                )
                s2T = s2_ps.tile([2, 128], F16, name=f"s2T_{t}", tag="s2T")
                nc.tensor.transpose(s2T, stat2, ident)
                nc.scalar.copy(corr2_all[:, t, :], s2T)

            def compute_tile(b, st, pools):
                (xtp, rotp, qkv_ps, tph_ps) = pools
                t = b * TPB + st
                rstd = rstd_all[:, t:t + 1]
                xT_t = xT_pref.pop(t)

                qkvp = qkv_ps.tile([128, 384], F32, name=f"qkvp_{b}_{st}", tag="qkvp")
                for c in range(KC):
                    nc.tensor.matmul(
                        qkvp, xT_t[:, c, :], wqkv_sb[:, c, :],
                        start=(c == 0), stop=False,
                    )
                nc.tensor.matmul(qkvp, corr2_all[:, t, :], corrw_sb,
                                 start=False, stop=True)

                # ---- rotary on q,k (cols 0:256) with rstd scale fused ----
                pv = qkvp[:, 0:256].rearrange("p (g d) -> p g d", g=4)
                cosb = _bc(tabs["cos"][:, st, :], 4)
                t1 = rotp.tile([128, 4, HD], F16, name=f"t1_{b}_{st}", tag="t1")
                nc.vector.scalar_tensor_tensor(
                    t1, pv, rstd, cosb,
                    op0=mybir.AluOpType.mult, op1=mybir.AluOpType.mult,
                )
                qr = rotp.tile([128, 4, HD], F16, name=f"qr_{b}_{st}", tag="qr")
                sx = tabs["sin"][:, st, :]
                # rotate-half: out[:, :, 0:32] <- -sin*pv[:, :, 32:64] etc.
                # (the sign lives in the sinx table); two 3D stt ops since
                # the verifier rejects 4D scalar_tensor_tensor inputs.
                for u in range(2):
                    out_u = qr[:, :, u * 32:(u + 1) * 32]
                    in_u = bass.AP(
                        tensor=pv.tensor,
                        offset=pv.offset + (1 - u) * 32,
                        ap=[pv.ap[0], [64, 4], [1, 32]],
                    )
                    sx_u = bass.AP(tensor=sx.tensor, offset=sx.offset + u * 32,
                                   ap=[sx.ap[0], [0, 4], [1, 32]])
                    nc.vector.scalar_tensor_tensor(
                        out_u, in_u, rstd, sx_u,
                        op0=mybir.AluOpType.mult, op1=mybir.AluOpType.mult,
                    )
                nc.vector.tensor_tensor(qr, qr, t1, op=mybir.AluOpType.add)

                # ---- transpose q,k to [hd, tok] and store ----
                tph = tph_ps.tile([64, 2, 256], F16, name=f"tph_{b}_{st}", tag="tph")
                for g in range(4):
                    nc.tensor.transpose(
                        tph[:, g // 2, (g % 2) * 128:(g % 2 + 1) * 128],
                        qr[:, g, :], ident,
                    )
                # engine choice per batch: Act has slack while batch-0
                # tiles run (few Exps yet), DVE has more slack later.
                q_eng = nc.vector.tensor_copy
                q_eng(
                    qT[b][0:64, :, st * 128:(st + 1) * 128],
                    tph[:, 0, :].rearrange("p (h f) -> p h f", h=2),
                )
                nc.scalar.copy(
                    kT[b][0:64, :, st * 128:(st + 1) * 128],
                    tph[:, 1, :].rearrange("p (h f) -> p h f", h=2),
                )
                # ---- v with rstd scale ----
                nc.scalar.mul(
                    v_sb[b][:, st, :, 0:64],
                    qkvp[:, 256:384].rearrange("p (h d) -> p h d", h=HPC),
                    rstd,
                )

            def attention_pair(b, h, gq, kp, ctxp, pools):
                (pp, rp, rbp, cstp, sc_ps, ctx_ps) = pools
                nkt = 4 * (gq + 1)
                kts = (2 * kp, 2 * kp + 1)
                qoffs = [
                    (kt - 4 * gq) * 128 if kt >= 4 * gq else 0 for kt in kts
                ]
                sc = sc_ps.tile([128, 2, 512], F32, name=f"sc_{b}_{h}_{gq}_{kp}", tag="sc")
                for i, kt in enumerate(kts):
                    nc.tensor.matmul(
                        sc[:, i, qoffs[i]:512],
                        kT[b][:, h, kt * 128:(kt + 1) * 128],
                        qT[b][:, h, gq * 512 + qoffs[i]:(gq + 1) * 512],
                        start=True, stop=True,
                    )
                pb = pp.tile([128, 2, 512], F16, name=f"pb_{b}_{h}_{gq}_{kp}", tag="pb")
                qmin = qoffs[0]
                nc.scalar.activation(
                    pb[:, :, qmin:512], sc[:, :, qmin:512],
                    mybir.ActivationFunctionType.Exp, scale=0.125,
                )
                if qoffs[1] > qoffs[0]:
                    nc.gpsimd.memset(pb[:, 1, qoffs[0]:qoffs[1]], 0.0)
                for i, kt in enumerate(kts):
                    if kt >= 4 * gq:
                        nc.gpsimd.affine_select(
                            out=pb[:, i, qoffs[i]:qoffs[i] + 128],
                            in_=pb[:, i, qoffs[i]:qoffs[i] + 128],
                            compare_op=mybir.AluOpType.is_ge,
                            fill=0.0, base=0,
                            pattern=[[1, 128]], channel_multiplier=-1,
                        )
                for i, kt in enumerate(kts):
                    nc.tensor.matmul(
                        ctxp[:, qoffs[i]:512],
                        v_sb[b][:, kt, h, :],
                        pb[:, i, qoffs[i]:512],
                        start=(kt == 0), stop=(kt == nkt - 1),
                    )

            def attention_finish(b, h, gq, ctxp, pools):
                (pp, rp, rbp, cstp, sc_ps, ctx_ps) = pools
                rin = rp.tile([1, 512], F32, name=f"rin_{b}_{h}_{gq}", tag="rin")
                with nc.allow_low_precision(reason="fp32r rounding within tolerance"):
                    nc.vector.reciprocal(rin, ctxp[64:65, :])
                rbb = rbp.tile([64, 512], F32, name=f"rbb_{b}_{h}_{gq}", tag="rbb")
                nc.gpsimd.partition_broadcast(rbb, rin)
                cst = cstp.tile([64, 512], F16, name=f"cst_{b}_{h}_{gq}", tag="cst")
                nc.vector.tensor_tensor(cst, ctxp[0:64, :], rbb, op=mybir.AluOpType.mult)
                nc.sync.dma_start(
                    out=ctx_local[h * 64:(h + 1) * 64,
                                  b * S + gq * 512: b * S + (gq + 1) * 512],
                    in_=cst,
                )

            def attention_unit(b, h, gq, pools):
                ctx_ps = pools[5]
                ctxp = ctx_ps.tile([65, 512], F32, name=f"ctxp_{b}_{h}_{gq}", tag="ctxp")
                for kp in range(2 * (gq + 1)):
                    attention_pair(b, h, gq, kp, ctxp, pools)
                attention_finish(b, h, gq, ctxp, pools)

            # ============ pools ============
            with (
                tc.tile_pool(name="xp", bufs=6) as xp,
                tc.tile_pool(name="statp", bufs=6) as statp,
                tc.tile_pool(name="xtp", bufs=6) as xtp,
                tc.tile_pool(name="rotp", bufs=3) as rotp,
                tc.tile_pool(name="s2_ps", bufs=1, space="PSUM") as s2_ps,
                tc.tile_pool(name="qkv_ps", bufs=2, space="PSUM") as qkv_ps,
                tc.tile_pool(name="tph_ps", bufs=1, space="PSUM") as tph_ps,
                tc.tile_pool(name="pp", bufs=3) as pp,
                tc.tile_pool(name="rp", bufs=2) as rp,
                tc.tile_pool(name="rbp", bufs=2) as rbp,
                tc.tile_pool(name="cstp", bufs=2) as cstp,
                tc.tile_pool(name="sc_ps", bufs=1, space="PSUM") as sc_ps,
                tc.tile_pool(name="ctx_ps", bufs=2, space="PSUM") as ctx_ps,
            ):
                papools = (xp, statp, s2_ps, xtp)
                cppools = (xtp, rotp, qkv_ps, tph_ps)
                atpools = (pp, rp, rbp, cstp, sc_ps, ctx_ps)

                # Fully woven schedule over 32 flat slots: pass A runs 2
                # tiles/slot ahead, computes go in flat tile order, and
                # attention units are emitted as soon as the k-tiles they
                # need exist, keeping Exp (Act) overlapped with gemm/rotary
                # work throughout. The final b1 units (gq=3, legal only at
                # the very end) run in a dedicated tail scope.
                uq = [(b, h, gq) for b in range(B) for gq in range(GQ)
                      for h in range(HPC)]
                tail_units = [(1, h, gq) for gq in (0, 1, 2, 3) for h in range(HPC)]
                uq = [u for u in uq if u not in tail_units]

                def legal(u, slot):
                    b, h, gq = u
                    return b * TPB + 4 * gq + 3 <= slot

                for t in range(2):
                    passA_tile(t, papools)
                for slot in range(NTILES):
                    for tt in (2 * slot + 2, 2 * slot + 3):
                        if tt < NTILES:
                            passA_tile(tt, papools)
                    compute_tile(slot // TPB, slot % TPB, cppools)
                    if slot == TPB - 1:
                        # output-projection weights; DMA overlaps compute
                        ow_sb = singles.tile([128, KC, H], F16)
                        nc.sync.dma_start(
                            out=ow_sb, in_=ow.rearrange("(c p) f -> p c f", p=128))
                    if uq and legal(uq[0], slot):
                        b, h, gq = uq.pop(0)
                        attention_unit(b, h, gq, atpools)
                for (b, h, gq) in uq:
                    attention_unit(b, h, gq, atpools)

            # tail: last b1 units, two at a time with pair-level round-robin
            # so one unit's Exp overlaps the other's score/ctx matmuls. The
            # output projection lives in the SAME pool scope: a scope
            # boundary here would insert a full cross-engine barrier + drain
            # (~10us PE idle) and reset the PE p-state ramp, halving the
            # projection matmul clock.
            with (
                tc.tile_pool(name="pp2", bufs=4) as pp2,
                tc.tile_pool(name="rp2", bufs=2) as rp2,
                tc.tile_pool(name="rbp2", bufs=2) as rbp2,
                tc.tile_pool(name="cstp2", bufs=2) as cstp2,
                tc.tile_pool(name="sc_ps2", bufs=2, space="PSUM") as sc_ps2,
                tc.tile_pool(name="ctx_ps2", bufs=2, space="PSUM") as ctx_ps2,
                tc.tile_pool(name="cap", bufs=1) as cap,
                tc.tile_pool(name="ostg", bufs=4) as ostg,
                tc.tile_pool(name="op_ps", bufs=2, space="PSUM") as op_ps,
            ):
                tailpools = (pp2, rp2, rbp2, cstp2, sc_ps2, ctx_ps2)
                rest = tail_units
                for w in range(0, len(rest), 2):
                    wave = rest[w:w + 2]
                    ctxps = {}
                    for (b, h, gq) in wave:
                        ctxps[(b, h, gq)] = ctx_ps2.tile(
                            [65, 512], F32, name=f"ctxp2_{b}_{h}_{gq}", tag="ctxp"
                        )
                    npairs = max(2 * (gq + 1) for (b, h, gq) in wave)
                    for kp in range(npairs):
                        for (b, h, gq) in wave:
                            if kp < 2 * (gq + 1):
                                attention_pair(b, h, gq, kp, ctxps[(b, h, gq)], tailpools)
                    for (b, h, gq) in wave:
                        attention_finish(b, h, gq, ctxps[(b, h, gq)], tailpools)

                # ============ AllGather ============
                if with_cc:
                    nc.gpsimd.collective_compute(
                        "AllGather", mybir.AluOpType.bypass,
                        replica_groups=[list(range(NCORES))],
                        ins=[ctx_local.opt()], outs=[ctx_all.opt()],
                    )

                # ============ output projection ============
                pid = nc.partition_id()
                base = pid * TPC
                ctx_r = ctx_all.rearrange("(c p) t -> p c t", p=128)
                ctxA = cap.tile([128, KC, TPC], F16, name="ctxA", tag="ctxA")
                # per-token-tile chunk DMAs on the Act queue: it drains right
                # after the last tail Exp, while the SP queue is still held
                # by the final cst DMAs' waits
                for tt in range(TPC // 128):
                    nc.scalar.dma_start(
                        out=ctxA[:, :, tt * 128:(tt + 1) * 128],
                        in_=ctx_r[:, :, bass.ds(base + tt * 128, 128)])
                # no bias matmul: ob is folded into the v-bias host-side
                # (softmax rows sum to 1, so a v-bias of ob @ ow^-1 emerges
                # from the projection as exactly +ob)
                for tt in range(TPC // 128):
                    ost = ostg.tile([128, H], F16, name=f"ost_{tt}", tag="ost")
                    for nh in range(2):
                        op = op_ps.tile([128, 512], F32, name=f"op_{tt}_{nh}", tag="op")
                        for c in range(KC):
                            nc.tensor.matmul(
                                op, ctxA[:, c, tt * 128:(tt + 1) * 128],
                                ow_sb[:, c, nh * 512:(nh + 1) * 512],
                                start=(c == 0), stop=(c == KC - 1),
                            )
                        (nc.scalar.copy if nh == 0 else nc.vector.tensor_copy)(
                            ost[:, nh * 512:(nh + 1) * 512], op)
                    nc.sync.dma_start(
                        out=out_slice[tt * 128:(tt + 1) * 128, :], in_=ost,
                    )
    nc.compile()
    return nc


def make_inputs(x, input_mask, norm_w, norm_b, attn_qkvw, attn_qkvb, attn_ow, attn_ob):
    """Host preprocessing -> list of per-core input dicts."""
    x = np.asarray(x, np.float32).reshape(T, H)
    input_mask = np.asarray(input_mask)
    norm_w = np.asarray(norm_w, np.float32)
    norm_b = np.asarray(norm_b, np.float32)
    attn_qkvw = np.asarray(attn_qkvw, np.float32)
    attn_qkvb = np.asarray(attn_qkvb, np.float32)
    attn_ow = np.asarray(attn_ow, np.float32)
    attn_ob = np.asarray(attn_ob, np.float32)

    wp = norm_w[:, None] * attn_qkvw                     # fold LN scale
    bp = attn_qkvb + norm_b @ attn_qkvw                  # fold LN shift
    # fold the output-projection bias into the v-bias: softmax rows sum to
    # 1, so adding c = ob @ ow^-1 to v adds exactly c @ ow = ob to the
    # final output after the projection.
    c_fold = np.linalg.solve(
        attn_ow.astype(np.float16).astype(np.float64).T,
        attn_ob.astype(np.float64),
    ).astype(np.float32)
    bp = bp.copy()
    bp[2 * H:] += c_fold

    pos = np.arange(S, dtype=np.float32)
    inv_freq = 1.0 / (10000.0 ** (np.arange(0, HD, 2, dtype=np.float32) / HD))
    freqs = pos[:, None] * inv_freq[None, :]             # [S, 32]
    cos_full = np.concatenate([np.cos(freqs)] * 2, -1)   # [S, 64]
    sin_full = np.concatenate([np.sin(freqs)] * 2, -1)
    sinx = sin_full.copy()
    sinx[:, :32] *= -1.0

    def tabify(a):  # [S, 64] -> [128, TPB, 64]
        return np.ascontiguousarray(
            a.reshape(TPB, 128, HD).swapaxes(0, 1).astype(np.float32)
        )

    cos_t = tabify(cos_full)
    sinx_t = tabify(sinx)

    # row 0: mask bias, pre-multiplied by 8 (exp applies scale=1/8), clamped
    # for fp16; row 1: ones (loaded into qT's 65th row)
    kbias_t = np.stack([
        np.clip((1.0 - input_mask.astype(np.float32)) * -10000.0 * 8.0,
                -60000.0, 0.0),
        np.ones((B, S), np.float32),
    ]).astype(np.float16)

    x16 = x.astype(np.float16)

    in_maps = []
    for c in range(NCORES):
        hs = slice(c * HPC * HD, (c + 1) * HPC * HD)     # this core's 128 cols
        wqkv_c = np.ascontiguousarray(
            np.concatenate([wp[:, hs], wp[:, H:][:, hs], wp[:, 2 * H:][:, hs]], axis=1)
        ).astype(np.float16)
        bqkv_c = np.concatenate([bp[hs], bp[H:][hs], bp[2 * H:][hs]])
        wsum_c = wqkv_c.astype(np.float32).sum(axis=0)   # sums of quantized W
        corr_c = np.ascontiguousarray(
            np.stack([wsum_c, bqkv_c]).astype(np.float32)
        )
        in_maps.append({
            "x": x16,
            "identm": np.eye(128, dtype=np.float16),
            "wqkv": wqkv_c,
            "corrw": corr_c,
            "cosx": cos_t, "sinx": sinx_t,
            "kbias": kbias_t,
            "ow": attn_ow.astype(np.float16),
        })
    return in_maps


_CACHE = {}


def _get_runner():
    """Build nc once and return a callable(in_maps) -> list of out dicts,
    reusing one jitted shard_map across calls."""
    if "runner" in _CACHE:
        return _CACHE["runner"]
    import jax
    import jax.numpy as jnp
    from jax.sharding import Mesh, PartitionSpec
    from jax.experimental.shard_map import shard_map
    from concourse import bass2jax
    from concourse import mybir as _mybir

    nc = build_nc()
    bass2jax.install_neuronx_cc_hook()

    partition_name = nc.partition_id_tensor.name if nc.partition_id_tensor else None
    in_names, out_names, out_avals = [], [], []
    for alloc in nc.m.functions[0].allocations:
        if not isinstance(_mybir.MemoryLocationSet, type) or not isinstance(alloc, _mybir.MemoryLocationSet):
            continue
        name = alloc.memorylocations[0].name
        if alloc.kind == "ExternalInput":
            if name != partition_name:
                in_names.append(name)
        elif alloc.kind == "ExternalOutput":
            out_names.append(name)
            out_avals.append(
                jax.core.ShapedArray(tuple(alloc.tensor_shape), _mybir.dt.np(alloc.dtype))
            )
    n_params = len(in_names)
    all_names = in_names + out_names
    if partition_name is not None:
        all_names.append(partition_name)

    def _body(*args):
        operands = list(args)
        if partition_name is not None:
            operands.append(bass2jax.partition_id_tensor())
        outs = bass2jax._bass_exec_p.bind(
            *operands,
            out_avals=tuple(out_avals),
            in_names=tuple(all_names),
            out_names=tuple(out_names),
            lowering_input_output_aliases=(),
            sim_require_finite=True,
            sim_require_nnan=True,
            nc=nc,
        )
        return tuple(outs)

    devices = jax.devices()[:NCORES]
    mesh = Mesh(np.asarray(devices), ("core",))
    n_outs = len(out_names)
    in_specs = (PartitionSpec("core"),) * (n_params + n_outs)
    out_specs = (PartitionSpec("core"),) * n_outs
    sharded = jax.jit(
        shard_map(_body, mesh=mesh, in_specs=in_specs, out_specs=out_specs,
                  check_rep=False),
        keep_unused=True,
    )

    from jax.sharding import NamedSharding
    shard = NamedSharding(mesh, PartitionSpec("core"))

    def to_device(in_maps):
        concat_in = [
            np.concatenate([np.asarray(in_maps[c][nm]) for c in range(NCORES)], axis=0)
            for nm in in_names
        ]
        concat_zeros = [
            np.zeros((NCORES * a.shape[0], *a.shape[1:]), a.dtype) for a in out_avals
        ]
        return [jax.device_put(a, shard) for a in concat_in + concat_zeros]

    def run_device(dev_args):
        out_arrs = sharded(*dev_args)
        jax.block_until_ready(out_arrs)
        return out_arrs

    def runner(in_maps):
        out_arrs = run_device(to_device(in_maps))
        return [
            {nm: np.asarray(out_arrs[i]).reshape(NCORES, *out_avals[i].shape)[c]
             for i, nm in enumerate(out_names)}
            for c in range(NCORES)
        ]

    runner.to_device = to_device
    runner.run_device = run_device
    _CACHE["runner"] = runner
    return runner


def kernel(**inputs) -> np.ndarray:
    in_maps = make_inputs(**inputs)
    runner = _get_runner()
    results = runner(in_maps)
    full = np.concatenate([results[c]["out_slice"] for c in range(NCORES)], axis=0)
    return full.reshape(B, S, H).astype(np.float32)

